# revision 62
# baseline (speedup 1.0000x reference)
"""Causal self-attention (B=4, T=2048, C=1024, H=16, D=64) on 8 trn2 cores.

Sharding: core c handles batch b = c//2 and head-group hg = c%2 (8 heads).
qkv projection is column-parallel, attention is head-parallel, out_proj is
row-parallel; the final 2-way partial-sum + bias happens on host.

Per-core device program, pipelined over head PAIRS so the qkv projection of
pair p+1 overlaps the attention of pair p:
  per pair p (heads 2p, 2p+1, living on partition halves 0-63 / 64-127):
    - qkvT = (W_slice.T @ x.T) + bias -> qT,kT [feat, tok], vT [feat, tok]
    - v2 = PE-transpose(vT) with interleaved ones columns (the ones column
      makes the attn@v matmul also emit the softmax denominator row)
    - per q-chunk: scoresT = kT.T@qT (causal-skipped + sliced), exp on ACT,
      128-wide triangle mask on DVE, ctxT_ext = [v|1].T @ exp in PSUM;
      denominators collected by DMA, batched reciprocal, broadcast across
      partitions by a partition-step-0 DMA, normalize ctx in place
  tail: y_partial = ctx_stacked.T @ W_out_slice -> DRAM
"""

import os
import sys

for _p in ("/opt/trn_rl_repo", "/root/.axon_site/_ro/trn_rl_repo"):
    if os.path.isdir(_p) and _p not in sys.path:
        sys.path.insert(0, _p)

import numpy as np

B, T, C = 4, 2048, 1024
H, D = 16, 64
NCORES = 8
HPC = 8          # heads per core
FQ = HPC * D     # 512 per-core q (=k=v) feature count
TK = T // 128    # 16 token tiles of 128
V2W = 130        # v2 per-ktile width: 64 + 1 + 64 + 1

_CACHE = {}


def _build_program():
    import concourse.bacc as bacc
    import concourse.tile as tile
    import concourse.mybir as mybir
    from contextlib import ExitStack

    f32 = mybir.dt.float32
    f32r = mybir.dt.float32r
    AF = mybir.ActivationFunctionType

    nc = bacc.Bacc("TRN2", target_bir_lowering=False, debug=False)

    x_t = nc.dram_tensor("x_t", [C, T], f32r, kind="ExternalInput").ap()
    w_s = nc.dram_tensor("w_s", [C, 3 * FQ], f32r, kind="ExternalInput").ap()
    b_s = nc.dram_tensor("b_s", [3 * FQ], f32, kind="ExternalInput").ap()
    w_o = nc.dram_tensor("w_o", [FQ, C], f32r, kind="ExternalInput").ap()
    tri_d = nc.dram_tensor("tri", [128, 128], f32, kind="ExternalInput").ap()
    idm_d = nc.dram_tensor("idm", [128, 128], f32r, kind="ExternalInput").ap()
    onec_d = nc.dram_tensor("onec", [128, 64], f32r, kind="ExternalInput").ap()
    y_d = nc.dram_tensor("y", [T, C], f32, kind="ExternalOutput").ap()

    with tile.TileContext(nc) as tc, ExitStack() as ctx:
        # ---- whole-kernel persistents ----
        pp = ctx.enter_context(tc.tile_pool(name="persist", bufs=1))
        ident = pp.tile([128, 128], f32r, tag="ident", name="ident")
        nc.sync.dma_start(out=ident, in_=idm_d)
        b_sb = pp.tile([128, 12], f32, tag="bias", name="b_sb")
        nc.sync.dma_start(out=b_sb, in_=b_s.rearrange("(f p) -> p f", p=128))
        tri_sb = pp.tile([128, 128], f32, tag="tri", name="tri_sb")
        nc.sync.dma_start(out=tri_sb, in_=tri_d)

        ctx_pool = ctx.enter_context(tc.tile_pool(name="ctxp", bufs=1))
        ctx_sb = [ctx_pool.tile([128, T], f32r, tag=f"c{i}", name=f"ctx_sb{i}")
                  for i in range(4)]

        # psum pools that span the whole pair pipeline (8 banks total:
        # qkv 1 + transpose 1 + scores 4 + ctx 2)
        ps1 = ctx.enter_context(tc.tile_pool(name="ps1", bufs=1, space="PSUM"))
        pst = ctx.enter_context(tc.tile_pool(name="pst", bufs=1, space="PSUM"))

        with tc.tile_pool(name="xres", bufs=1) as xp, \
             tc.tile_pool(name="wstr", bufs=2) as wp, \
             tc.tile_pool(name="qkq", bufs=2) as qkq, \
             tc.tile_pool(name="vTq", bufs=2) as vTq, \
             tc.tile_pool(name="v2q", bufs=2) as v2q, \
             tc.tile_pool(name="collp", bufs=3) as coll_pool, \
             tc.tile_pool(name="bcq", bufs=3) as bcq, \
             tc.tile_pool(name="dscr", bufs=4, space="DRAM") as dscr, \
             tc.tile_pool(name="stg", bufs=4) as stg, \
             tc.tile_pool(name="esb", bufs=3) as esb, \
             tc.tile_pool(name="scps", bufs=2, space="PSUM") as scps, \
             tc.tile_pool(name="cxps", bufs=2, space="PSUM") as cxps:

            x_sb = [xp.tile([128, T], f32r, tag=f"x{ks}", name=f"x_sb{ks}")
                    for ks in range(8)]
            # chunked tcn-outer so the first token-chunk's 8 k-slices arrive
            # quickly and the first psum accumulation can start early
            for tcn in range(4):
                for ks in range(8):
                    nc.sync.dma_start(
                        out=x_sb[ks][:, tcn * 512:(tcn + 1) * 512],
                        in_=x_t[ks * 128:(ks + 1) * 128,
                                tcn * 512:(tcn + 1) * 512])

            for pair in range(4):
                # ---- qkv projection for this pair (v first, then q, k) ----
                qp = qkq.tile([128, T], f32r, tag="qp", name="q_p")
                kp = qkq.tile([128, T], f32r, tag="kp", name="k_p")
                vT = vTq.tile([128, T], f32r, tag="vT", name="vT_p")
                v2 = v2q.tile([128, TK * V2W], f32r, tag="v2", name="v2_p")
                v2v = v2.rearrange("p (t w) -> p t w", w=V2W)
                onec3 = onec_d[:, 0:16].rearrange("p (t o) -> p t o", o=1)
                nc.sync.dma_start(out=v2v[:, :, 64:65], in_=onec3)
                nc.sync.dma_start(out=v2v[:, :, 129:130], in_=onec3)

                for ft, dest in ((8 + pair, vT), (pair, qp), (4 + pair, kp)):
                    wt = [wp.tile([128, 128], f32r, tag=f"w{ks}",
                                  name=f"wt{ks}") for ks in range(8)]
                    for ks in range(8):
                        nc.gpsimd.dma_start(
                            out=wt[ks],
                            in_=w_s[ks * 128:(ks + 1) * 128,
                                    ft * 128:(ft + 1) * 128])
                    for tcn in range(4):
                        # double-buffer the qkv psum by ping-ponging between
                        # the two single-slot pools; the transpose slot is
                        # only contended during the v third
                        if dest is vT or tcn % 2 == 0:
                            ps = ps1.tile([128, 512], f32, tag="qkvps",
                                          name="qkv_ps")
                        else:
                            ps = pst.tile([128, 512], f32, tag="tp",
                                          name="qkv_ps2")
                        for ks in range(8):
                            nc.tensor.matmul(
                                ps, lhsT=wt[ks],
                                rhs=x_sb[ks][:, tcn * 512:(tcn + 1) * 512],
                                start=(ks == 0), stop=(ks == 7))
                        # evacuate with fused bias add: out = psum + b
                        nc.scalar.activation(
                            dest[:, tcn * 512:(tcn + 1) * 512], ps,
                            AF.Identity, bias=b_sb[:, ft:ft + 1], scale=1.0)
                        if dest is vT:
                            # v2 build interleaved: transpose the 4 ktiles of
                            # this freshly written v token-chunk
                            for kt in range(4 * tcn, 4 * tcn + 4):
                                pt = pst.tile([128, 128], f32r, tag="tp",
                                              name="tr_ps")
                                nc.tensor.transpose(
                                    pt, vT[:, kt * 128:(kt + 1) * 128], ident)
                                base = kt * V2W
                                nc.vector.tensor_copy(
                                    v2[:, base:base + 64], pt[:, 0:64])
                                nc.vector.tensor_copy(
                                    v2[:, base + 65:base + 129],
                                    pt[:, 64:128])

                # ---- attention for this pair, all 4 q-chunks ----
                for qc in range(4):
                    collq = coll_pool.tile([2, 512], f32, tag="cq",
                                           name="collq")
                    collrq = coll_pool.tile([2, 512], f32r, tag="cr",
                                            name="collrq")
                    ngr = 2 * (qc + 1)      # groups of 2 ktiles
                    c_ext = [cxps.tile([65, 512], f32, tag="cext",
                                       name="c_ext") for _ in range(2)]
                    for g in range(ngr):
                        diag = g >= ngr - 2
                        m = g - (ngr - 2)
                        scp = [scps.tile([128, 1024], f32, tag="sc",
                                         name="sc_ps") for _ in range(2)]
                        for j in range(2):
                            kt = 2 * g + j
                            roff = (2 * m + j) * 128 if diag else 0
                            for side in range(2):
                                poff = side * 64
                                nc.tensor.matmul(
                                    scp[side][:, j * 512 + roff:
                                              (j + 1) * 512],
                                    lhsT=kp[poff:poff + 64,
                                            kt * 128:(kt + 1) * 128],
                                    rhs=qp[poff:poff + 64,
                                           qc * 512 + roff:(qc + 1) * 512],
                                    start=True, stop=True)
                        ee = []
                        for side in range(2):
                            e = esb.tile([128, 1024], f32r, tag="e",
                                         name="e_sb")
                            if diag and m == 1:
                                # only ktiles r=2,3 live here; exp just the
                                # valid column ranges
                                nc.scalar.activation(
                                    e[:, 256:512], scp[side][:, 256:512],
                                    AF.Exp, scale=0.125)
                                nc.scalar.activation(
                                    e[:, 896:1024], scp[side][:, 896:1024],
                                    AF.Exp, scale=0.125)
                            else:
                                nc.scalar.activation(e, scp[side], AF.Exp,
                                                     scale=0.125)
                            if diag:
                                # in-tile causal boundary: 128-wide triangle
                                # per diagonal ktile
                                for j in range(2):
                                    r = 2 * m + j
                                    c0 = j * 512 + r * 128
                                    nc.vector.tensor_mul(
                                        e[:, c0:c0 + 128],
                                        e[:, c0:c0 + 128], tri_sb)
                            ee.append(e)
                        for j in range(2):
                            kt = 2 * g + j
                            r = 2 * m + j
                            roff = r * 128 if diag else 0
                            for side in range(2):
                                vb = kt * V2W + side * 65
                                nc.tensor.matmul(
                                    c_ext[side][:, roff:512],
                                    lhsT=v2[:, vb:vb + 65],
                                    rhs=ee[side][:, j * 512 + roff:
                                                 (j + 1) * 512],
                                    start=(g == 0 and j == 0),
                                    stop=(g == ngr - 1 and j == 1))
                    for side in range(2):
                        poff = side * 64
                        # engine APs need 32-aligned partition bases, so the
                        # denominator row (psum partition 64) is staged on
                        # partition 64 and moved to the collector row by DMA
                        dst = stg.tile([65, 512], f32, tag="dstage",
                                       name="dstage")
                        nc.vector.tensor_copy(dst[64:65, :],
                                              c_ext[side][64:65, :])
                        nc.sync.dma_start(out=collq[side:side + 1, :],
                                          in_=dst[64:65, :])
                        nc.vector.tensor_copy(
                            ctx_sb[pair][poff:poff + 64,
                                         qc * 512:(qc + 1) * 512],
                            c_ext[side][0:64, :])
                    # normalize: batched reciprocal of both heads' rows, then
                    # partition-broadcast each row by a step-0 DMA
                    with nc.allow_low_precision(reason="f32r == f32 storage"):
                        nc.vector.reciprocal(collrq, collq)
                    dsc = dscr.tile([2, 512], f32r, tag="ds", name="dsc")
                    nc.scalar.dma_start(out=dsc, in_=collrq)
                    # one [128,512] tile, each head's reciprocal row broadcast
                    # over its own partition half so the multiply's operand
                    # base partitions match
                    bcast = bcq.tile([128, 512], f32r, tag="bc", name="bcast")
                    for side in range(2):
                        nc.scalar.dma_start(
                            out=bcast[side * 64:(side + 1) * 64, :],
                            in_=dsc[side:side + 1, :].to_broadcast(
                                [64, 512]))
                    for side in range(2):
                        poff = side * 64
                        cslice = ctx_sb[pair][poff:poff + 64,
                                              qc * 512:(qc + 1) * 512]
                        nc.vector.tensor_mul(cslice, cslice,
                                             bcast[poff:poff + 64, :])

        # ---------------- tail: out projection ----------------
        with tc.tile_pool(name="wop", bufs=1) as wop, \
             tc.tile_pool(name="yps", bufs=4, space="PSUM") as yps, \
             tc.tile_pool(name="ysbp", bufs=4) as ysbp:
            w_o_sb = [wop.tile([128, C], f32r, tag=f"wo{i}", name=f"wo_sb{i}")
                      for i in range(4)]
            for f in range(4):
                nc.gpsimd.dma_start(out=w_o_sb[f],
                                    in_=w_o[f * 128:(f + 1) * 128, :])
            for tt in range(TK):
                for oc in range(2):
                    yp = yps.tile([128, 512], f32, tag="yp", name="y_ps")
                    for f in range(4):
                        nc.tensor.matmul(
                            yp, lhsT=ctx_sb[f][:, tt * 128:(tt + 1) * 128],
                            rhs=w_o_sb[f][:, oc * 512:(oc + 1) * 512],
                            start=(f == 0), stop=(f == 3))
                    ysb = ysbp.tile([128, 512], f32, tag="ysb", name="y_sb")
                    if oc == 0:
                        nc.scalar.activation(ysb, yp, AF.Copy)
                    else:
                        nc.vector.tensor_copy(ysb, yp)
                    nc.sync.dma_start(
                        out=y_d[tt * 128:(tt + 1) * 128,
                                oc * 512:(oc + 1) * 512],
                        in_=ysb)

    nc.compile()
    return nc


def _host_inputs(x, w_qkv, b_qkv, w_out):
    """Build the 8 per-core input maps."""
    tri = (np.arange(128)[:, None] <= np.arange(128)[None, :]).astype(
        np.float32)

    xt = [np.ascontiguousarray(x[b].T) for b in range(B)]      # [C, T] each
    in_maps = []
    for core in range(NCORES):
        b, hg = core // 2, core % 2
        cs = slice(hg * FQ, (hg + 1) * FQ)
        w_slice = np.concatenate(
            [w_qkv[:, cs], w_qkv[:, C + hg * FQ: C + (hg + 1) * FQ],
             w_qkv[:, 2 * C + hg * FQ: 2 * C + (hg + 1) * FQ]], axis=1)
        b_slice = np.concatenate(
            [b_qkv[cs], b_qkv[C + hg * FQ: C + (hg + 1) * FQ],
             b_qkv[2 * C + hg * FQ: 2 * C + (hg + 1) * FQ]])
        in_maps.append({
            "x_t": xt[b],
            "w_s": np.ascontiguousarray(w_slice),
            "b_s": np.ascontiguousarray(b_slice),
            "w_o": np.ascontiguousarray(w_out[hg * FQ:(hg + 1) * FQ, :]),
            "tri": tri,
            "idm": np.eye(128, dtype=np.float32),
            "onec": np.ones((128, 64), dtype=np.float32),
        })
    return in_maps


def get_program():
    if "nc" not in _CACHE:
        _CACHE["nc"] = _build_program()
    return _CACHE["nc"]


def kernel(x, w_qkv, b_qkv, w_out, b_out):
    from concourse.bass_utils import run_bass_kernel_spmd

    x = np.asarray(x, dtype=np.float32)
    w_qkv = np.asarray(w_qkv, dtype=np.float32)
    b_qkv = np.asarray(b_qkv, dtype=np.float32)
    w_out = np.asarray(w_out, dtype=np.float32)
    b_out = np.asarray(b_out, dtype=np.float32)

    nc = get_program()
    in_maps = _host_inputs(x, w_qkv, b_qkv, w_out)
    res = run_bass_kernel_spmd(nc, in_maps, core_ids=list(range(NCORES)))

    out = np.empty((B, T, C), dtype=np.float32)
    for b in range(B):
        out[b] = res.results[2 * b]["y"] + res.results[2 * b + 1]["y"] + b_out
    return out


# revision 63
# speedup vs baseline: 1.0011x; 1.0011x over previous
"""Causal self-attention (B=4, T=2048, C=1024, H=16, D=64) on 8 trn2 cores.

Sharding: core c handles batch b = c//2 and head-group hg = c%2 (8 heads).
qkv projection is column-parallel, attention is head-parallel, out_proj is
row-parallel; the final 2-way partial-sum + bias happens on host.

Per-core device program, pipelined over head PAIRS so the qkv projection of
pair p+1 overlaps the attention of pair p:
  per pair p (heads 2p, 2p+1, living on partition halves 0-63 / 64-127):
    - qkvT = (W_slice.T @ x.T) + bias -> qT,kT [feat, tok], vT [feat, tok]
    - v2 = PE-transpose(vT) with interleaved ones columns (the ones column
      makes the attn@v matmul also emit the softmax denominator row)
    - per q-chunk: scoresT = kT.T@qT (causal-skipped + sliced), exp on ACT,
      128-wide triangle mask on DVE, ctxT_ext = [v|1].T @ exp in PSUM;
      denominators collected by DMA, batched reciprocal, broadcast across
      partitions by a partition-step-0 DMA, normalize ctx in place
  tail: y_partial = ctx_stacked.T @ W_out_slice -> DRAM
"""

import os
import sys

for _p in ("/opt/trn_rl_repo", "/root/.axon_site/_ro/trn_rl_repo"):
    if os.path.isdir(_p) and _p not in sys.path:
        sys.path.insert(0, _p)

import numpy as np

B, T, C = 4, 2048, 1024
H, D = 16, 64
NCORES = 8
HPC = 8          # heads per core
FQ = HPC * D     # 512 per-core q (=k=v) feature count
TK = T // 128    # 16 token tiles of 128
V2W = 130        # v2 per-ktile width: 64 + 1 + 64 + 1

_CACHE = {}


def _build_program():
    import concourse.bacc as bacc
    import concourse.tile as tile
    import concourse.mybir as mybir
    from contextlib import ExitStack

    f32 = mybir.dt.float32
    f32r = mybir.dt.float32r
    AF = mybir.ActivationFunctionType

    nc = bacc.Bacc("TRN2", target_bir_lowering=False, debug=False)

    x_t = nc.dram_tensor("x_t", [C, T], f32r, kind="ExternalInput").ap()
    w_s = nc.dram_tensor("w_s", [C, 3 * FQ], f32r, kind="ExternalInput").ap()
    b_s = nc.dram_tensor("b_s", [3 * FQ], f32, kind="ExternalInput").ap()
    w_o = nc.dram_tensor("w_o", [FQ, C], f32r, kind="ExternalInput").ap()
    tri_d = nc.dram_tensor("tri", [128, 128], f32, kind="ExternalInput").ap()
    idm_d = nc.dram_tensor("idm", [128, 128], f32r, kind="ExternalInput").ap()
    onec_d = nc.dram_tensor("onec", [128, 64], f32r, kind="ExternalInput").ap()
    y_d = nc.dram_tensor("y", [T, C], f32, kind="ExternalOutput").ap()

    with tile.TileContext(nc) as tc, ExitStack() as ctx:
        # ---- whole-kernel persistents ----
        pp = ctx.enter_context(tc.tile_pool(name="persist", bufs=1))
        ident = pp.tile([128, 128], f32r, tag="ident", name="ident")
        nc.sync.dma_start(out=ident, in_=idm_d)
        b_sb = pp.tile([128, 12], f32, tag="bias", name="b_sb")
        nc.sync.dma_start(out=b_sb, in_=b_s.rearrange("(f p) -> p f", p=128))
        tri_sb = pp.tile([128, 128], f32, tag="tri", name="tri_sb")
        nc.sync.dma_start(out=tri_sb, in_=tri_d)

        ctx_pool = ctx.enter_context(tc.tile_pool(name="ctxp", bufs=1))
        ctx_sb = [ctx_pool.tile([128, T], f32r, tag=f"c{i}", name=f"ctx_sb{i}")
                  for i in range(4)]

        # psum pools that span the whole pair pipeline (8 banks total:
        # qkv 1 + transpose 1 + scores 4 + ctx 2)
        ps1 = ctx.enter_context(tc.tile_pool(name="ps1", bufs=1, space="PSUM"))
        pst = ctx.enter_context(tc.tile_pool(name="pst", bufs=1, space="PSUM"))

        with tc.tile_pool(name="xres", bufs=1) as xp, \
             tc.tile_pool(name="wstr", bufs=2) as wp, \
             tc.tile_pool(name="qkq", bufs=2) as qkq, \
             tc.tile_pool(name="vTq", bufs=2) as vTq, \
             tc.tile_pool(name="v2q", bufs=2) as v2q, \
             tc.tile_pool(name="collp", bufs=3) as coll_pool, \
             tc.tile_pool(name="bcq", bufs=3) as bcq, \
             tc.tile_pool(name="dscr", bufs=4, space="DRAM") as dscr, \
             tc.tile_pool(name="stg", bufs=4) as stg, \
             tc.tile_pool(name="esb", bufs=3) as esb, \
             tc.tile_pool(name="scps", bufs=2, space="PSUM") as scps, \
             tc.tile_pool(name="cxps", bufs=2, space="PSUM") as cxps:

            x_sb = [xp.tile([128, T], f32r, tag=f"x{ks}", name=f"x_sb{ks}")
                    for ks in range(8)]
            # chunked tcn-outer so the first token-chunk's 8 k-slices arrive
            # quickly and the first psum accumulation can start early
            for tcn in range(4):
                for ks in range(8):
                    nc.sync.dma_start(
                        out=x_sb[ks][:, tcn * 512:(tcn + 1) * 512],
                        in_=x_t[ks * 128:(ks + 1) * 128,
                                tcn * 512:(tcn + 1) * 512])

            for pair in range(4):
                # ---- qkv projection for this pair (v first, then q, k) ----
                qp = qkq.tile([128, T], f32r, tag="qp", name="q_p")
                kp = qkq.tile([128, T], f32r, tag="kp", name="k_p")
                vT = vTq.tile([128, T], f32r, tag="vT", name="vT_p")
                v2 = v2q.tile([128, TK * V2W], f32r, tag="v2", name="v2_p")
                v2v = v2.rearrange("p (t w) -> p t w", w=V2W)
                onec3 = onec_d[:, 0:16].rearrange("p (t o) -> p t o", o=1)
                nc.sync.dma_start(out=v2v[:, :, 64:65], in_=onec3)
                nc.sync.dma_start(out=v2v[:, :, 129:130], in_=onec3)

                for ft, dest in ((8 + pair, vT), (pair, qp), (4 + pair, kp)):
                    wt = [wp.tile([128, 128], f32r, tag=f"w{ks}",
                                  name=f"wt{ks}") for ks in range(8)]
                    for ks in range(8):
                        nc.gpsimd.dma_start(
                            out=wt[ks],
                            in_=w_s[ks * 128:(ks + 1) * 128,
                                    ft * 128:(ft + 1) * 128])
                    for tcn in range(4):
                        # double-buffer the qkv psum by ping-ponging between
                        # the two single-slot pools; the transpose slot is
                        # only contended during the v third
                        if dest is vT or tcn % 2 == 0:
                            ps = ps1.tile([128, 512], f32, tag="qkvps",
                                          name="qkv_ps")
                        else:
                            ps = pst.tile([128, 512], f32, tag="tp",
                                          name="qkv_ps2")
                        for ks in range(8):
                            nc.tensor.matmul(
                                ps, lhsT=wt[ks],
                                rhs=x_sb[ks][:, tcn * 512:(tcn + 1) * 512],
                                start=(ks == 0), stop=(ks == 7))
                        # evacuate with fused bias add: out = psum + b
                        nc.scalar.activation(
                            dest[:, tcn * 512:(tcn + 1) * 512], ps,
                            AF.Identity, bias=b_sb[:, ft:ft + 1], scale=1.0)
                        if dest is vT:
                            # v2 build interleaved: transpose the 4 ktiles of
                            # this freshly written v token-chunk
                            for kt in range(4 * tcn, 4 * tcn + 4):
                                pt = pst.tile([128, 128], f32r, tag="tp",
                                              name="tr_ps")
                                nc.tensor.transpose(
                                    pt, vT[:, kt * 128:(kt + 1) * 128], ident)
                                base = kt * V2W
                                nc.vector.tensor_copy(
                                    v2[:, base:base + 64], pt[:, 0:64])
                                nc.vector.tensor_copy(
                                    v2[:, base + 65:base + 129],
                                    pt[:, 64:128])

                # ---- attention for this pair, all 4 q-chunks ----
                for qc in range(4):
                    collq = coll_pool.tile([2, 512], f32, tag="cq",
                                           name="collq")
                    collrq = coll_pool.tile([2, 512], f32r, tag="cr",
                                            name="collrq")
                    ngr = 2 * (qc + 1)      # groups of 2 ktiles
                    c_ext = [cxps.tile([65, 512], f32, tag="cext",
                                       name="c_ext") for _ in range(2)]
                    for g in range(ngr):
                        diag = g >= ngr - 2
                        m = g - (ngr - 2)
                        scp = [scps.tile([128, 1024], f32, tag="sc",
                                         name="sc_ps") for _ in range(2)]
                        for j in range(2):
                            kt = 2 * g + j
                            roff = min((2 * m + j) * 128, 256) if diag else 0
                            for side in range(2):
                                poff = side * 64
                                nc.tensor.matmul(
                                    scp[side][:, j * 512 + roff:
                                              (j + 1) * 512],
                                    lhsT=kp[poff:poff + 64,
                                            kt * 128:(kt + 1) * 128],
                                    rhs=qp[poff:poff + 64,
                                           qc * 512 + roff:(qc + 1) * 512],
                                    start=True, stop=True)
                        ee = []
                        for side in range(2):
                            e = esb.tile([128, 1024], f32r, tag="e",
                                         name="e_sb")
                            if diag and m == 1:
                                # only ktiles r=2,3 live here; exp just the
                                # valid column ranges
                                nc.scalar.activation(
                                    e[:, 256:512], scp[side][:, 256:512],
                                    AF.Exp, scale=0.125)
                                nc.scalar.activation(
                                    e[:, 896:1024], scp[side][:, 896:1024],
                                    AF.Exp, scale=0.125)
                            else:
                                nc.scalar.activation(e, scp[side], AF.Exp,
                                                     scale=0.125)
                            if diag:
                                # in-tile causal boundary: 128-wide triangle
                                # per diagonal ktile
                                for j in range(2):
                                    r = 2 * m + j
                                    c0 = j * 512 + r * 128
                                    nc.vector.tensor_mul(
                                        e[:, c0:c0 + 128],
                                        e[:, c0:c0 + 128], tri_sb)
                            ee.append(e)
                        for j in range(2):
                            kt = 2 * g + j
                            r = 2 * m + j
                            roff = r * 128 if diag else 0
                            for side in range(2):
                                vb = kt * V2W + side * 65
                                nc.tensor.matmul(
                                    c_ext[side][:, roff:512],
                                    lhsT=v2[:, vb:vb + 65],
                                    rhs=ee[side][:, j * 512 + roff:
                                                 (j + 1) * 512],
                                    start=(g == 0 and j == 0),
                                    stop=(g == ngr - 1 and j == 1))
                    for side in range(2):
                        poff = side * 64
                        # engine APs need 32-aligned partition bases, so the
                        # denominator row (psum partition 64) is staged on
                        # partition 64 and moved to the collector row by DMA
                        dst = stg.tile([65, 512], f32, tag="dstage",
                                       name="dstage")
                        nc.vector.tensor_copy(dst[64:65, :],
                                              c_ext[side][64:65, :])
                        nc.sync.dma_start(out=collq[side:side + 1, :],
                                          in_=dst[64:65, :])
                        nc.vector.tensor_copy(
                            ctx_sb[pair][poff:poff + 64,
                                         qc * 512:(qc + 1) * 512],
                            c_ext[side][0:64, :])
                    # normalize: batched reciprocal of both heads' rows, then
                    # partition-broadcast each row by a step-0 DMA
                    with nc.allow_low_precision(reason="f32r == f32 storage"):
                        nc.vector.reciprocal(collrq, collq)
                    dsc = dscr.tile([2, 512], f32r, tag="ds", name="dsc")
                    nc.scalar.dma_start(out=dsc, in_=collrq)
                    # one [128,512] tile, each head's reciprocal row broadcast
                    # over its own partition half so the multiply's operand
                    # base partitions match
                    bcast = bcq.tile([128, 512], f32r, tag="bc", name="bcast")
                    for side in range(2):
                        nc.scalar.dma_start(
                            out=bcast[side * 64:(side + 1) * 64, :],
                            in_=dsc[side:side + 1, :].to_broadcast(
                                [64, 512]))
                    for side in range(2):
                        poff = side * 64
                        cslice = ctx_sb[pair][poff:poff + 64,
                                              qc * 512:(qc + 1) * 512]
                        nc.vector.tensor_mul(cslice, cslice,
                                             bcast[poff:poff + 64, :])

        # ---------------- tail: out projection ----------------
        with tc.tile_pool(name="wop", bufs=1) as wop, \
             tc.tile_pool(name="yps", bufs=4, space="PSUM") as yps, \
             tc.tile_pool(name="ysbp", bufs=4) as ysbp:
            w_o_sb = [wop.tile([128, C], f32r, tag=f"wo{i}", name=f"wo_sb{i}")
                      for i in range(4)]
            for f in range(4):
                nc.gpsimd.dma_start(out=w_o_sb[f],
                                    in_=w_o[f * 128:(f + 1) * 128, :])
            for tt in range(TK):
                for oc in range(2):
                    yp = yps.tile([128, 512], f32, tag="yp", name="y_ps")
                    for f in range(4):
                        nc.tensor.matmul(
                            yp, lhsT=ctx_sb[f][:, tt * 128:(tt + 1) * 128],
                            rhs=w_o_sb[f][:, oc * 512:(oc + 1) * 512],
                            start=(f == 0), stop=(f == 3))
                    ysb = ysbp.tile([128, 512], f32, tag="ysb", name="y_sb")
                    if oc == 0:
                        nc.scalar.activation(ysb, yp, AF.Copy)
                    else:
                        nc.vector.tensor_copy(ysb, yp)
                    nc.sync.dma_start(
                        out=y_d[tt * 128:(tt + 1) * 128,
                                oc * 512:(oc + 1) * 512],
                        in_=ysb)

    nc.compile()
    return nc


def _host_inputs(x, w_qkv, b_qkv, w_out):
    """Build the 8 per-core input maps."""
    tri = (np.arange(128)[:, None] <= np.arange(128)[None, :]).astype(
        np.float32)

    xt = [np.ascontiguousarray(x[b].T) for b in range(B)]      # [C, T] each
    in_maps = []
    for core in range(NCORES):
        b, hg = core // 2, core % 2
        cs = slice(hg * FQ, (hg + 1) * FQ)
        w_slice = np.concatenate(
            [w_qkv[:, cs], w_qkv[:, C + hg * FQ: C + (hg + 1) * FQ],
             w_qkv[:, 2 * C + hg * FQ: 2 * C + (hg + 1) * FQ]], axis=1)
        b_slice = np.concatenate(
            [b_qkv[cs], b_qkv[C + hg * FQ: C + (hg + 1) * FQ],
             b_qkv[2 * C + hg * FQ: 2 * C + (hg + 1) * FQ]])
        in_maps.append({
            "x_t": xt[b],
            "w_s": np.ascontiguousarray(w_slice),
            "b_s": np.ascontiguousarray(b_slice),
            "w_o": np.ascontiguousarray(w_out[hg * FQ:(hg + 1) * FQ, :]),
            "tri": tri,
            "idm": np.eye(128, dtype=np.float32),
            "onec": np.ones((128, 64), dtype=np.float32),
        })
    return in_maps


def get_program():
    if "nc" not in _CACHE:
        _CACHE["nc"] = _build_program()
    return _CACHE["nc"]


def kernel(x, w_qkv, b_qkv, w_out, b_out):
    from concourse.bass_utils import run_bass_kernel_spmd

    x = np.asarray(x, dtype=np.float32)
    w_qkv = np.asarray(w_qkv, dtype=np.float32)
    b_qkv = np.asarray(b_qkv, dtype=np.float32)
    w_out = np.asarray(w_out, dtype=np.float32)
    b_out = np.asarray(b_out, dtype=np.float32)

    nc = get_program()
    in_maps = _host_inputs(x, w_qkv, b_qkv, w_out)
    res = run_bass_kernel_spmd(nc, in_maps, core_ids=list(range(NCORES)))

    out = np.empty((B, T, C), dtype=np.float32)
    for b in range(B):
        out[b] = res.results[2 * b]["y"] + res.results[2 * b + 1]["y"] + b_out
    return out
